# revision 23
# baseline (speedup 1.0000x reference)
"""Multi-head attention Trainium2 kernel, 8-way sharded, mask-compacted keys.

Problem: x[4,2048,1024] -> qkv proj (w_qkv [3072,1024]) -> 16-head attention
with key-padding mask -> tail proj (w_tail [1024,1024]) + b_tail.

Sharding: 8 shards = 4 batches x 2 head-groups (8 heads each). Each core
computes, for its (batch b, head-group hg):
  - q projection of x[b] (all T tokens) for its 8 heads
  - k/v projections of the mask-COMPACTED tokens of x[b] (keys with mask=0
    contribute exp(-inf)=0 to softmax, so they are dropped host-side and
    the key axis padded to KPAD, a multiple of 128; pads get bias -8e9)
  - [T x KPAD] masked attention per head
  - partial tail matmul y_part = attn_cat @ w_tail[:, cat_slice].T
Host unshards: out[b] = y_part[2b] + y_part[2b+1] + b_tail.  No collectives.

Engine strategy (trace-driven):
  - phase 1 (projections) and phase 3 (tail) are PE-dense: bf16 operands
    (1 cyc/row, background weight loads), PSUM->SBUF copies alternate
    between DVE and ACT so neither serializes the PE.
  - phase 2 (attention) is paced by ACT exp ([128,1024] tiles); PE operands
    stay float32r: the serial 4-byte weight load pads PE occupancy to
    ~match ACT, keeping the HAM clock at 8/8 (bf16 here made the PE idle
    23% per kb and the HAM halved the clock for ~180us).
"""

import time as _time

import numpy as np
from contextlib import ExitStack

import concourse.bass as bass
import concourse.mybir as mybir
import concourse.tile as tile
from concourse.bass_utils import run_bass_kernel_spmd

# ---------------------------------------------------------------------------
# walrus in this env accepts at most 2 sync waits per instruction; Tile's
# scheduler emits up to 10. Post-pass: peel excess waits onto same-engine
# NoOps inserted immediately before the offending instruction (same engine
# stream position => identical synchronization semantics).
MAX_WAITS = 1


def split_excess_waits(nc):
    for fn in nc.m.functions:
        for bb in fn.blocks:
            insts = list(bb.instructions)
            out = []
            changed = False
            for inst in insts:
                si = inst.sync_info
                waits = list(si.on_wait) if si is not None else []
                if len(waits) > MAX_WAITS:
                    extra = waits[:-MAX_WAITS]
                    for ci in range(0, len(extra), MAX_WAITS):
                        chunk = extra[ci:ci + MAX_WAITS]
                        nop = mybir.InstNoOp(
                            name=f"{inst.name}-ws{ci}", ins=[], outs=[])
                        nop.engine = inst.engine
                        nop.sync_info = mybir.SyncInfo(
                            on_wait=chunk, on_update=[])
                        out.append(nop)
                    inst.sync_info = mybir.SyncInfo(
                        on_wait=waits[-MAX_WAITS:],
                        on_update=list(si.on_update))
                    changed = True
                out.append(inst)
            if changed:
                bb.instructions = out
# ---------------------------------------------------------------------------

D_MODEL = 1024
N_HEAD = 16
D_HEAD = 64
BN, T = 4, 2048
HPC = 8                      # heads per core
NPAIR = HPC // 2             # head pairs (q/k tiles hold 2 heads)
CAT = HPC * D_HEAD           # 512 per-core tail contraction
NTB = T // 128               # 16 query-token blocks
QH = T // 2                  # 1024, q processed in two halves
KC = D_MODEL // 128          # 8 contraction chunks
F32 = mybir.dt.float32
F32R = mybir.dt.float32r
BF16 = mybir.dt.bfloat16
I32 = mybir.dt.int32


def build_nc(kpad, split_waits=True):
    assert kpad % 128 == 0 and 128 <= kpad <= T
    NKB = kpad // 128        # key blocks
    LAG = min(4, NKB - 1) if NKB > 1 else 0
    # k-projection chunks: (start, width), width 512 or the tail remainder
    KCH = [(c * 512, min(512, kpad - c * 512))
           for c in range((kpad + 511) // 512)]

    nc = bass.Bass()
    xT = nc.declare_dram_parameter("xT", [D_MODEL, T], BF16, isOutput=False)
    xkT = nc.declare_dram_parameter("xkT", [D_MODEL, kpad], BF16, isOutput=False)
    wqT = nc.declare_dram_parameter("wqT", [D_MODEL, CAT], BF16, isOutput=False)
    wkT = nc.declare_dram_parameter("wkT", [D_MODEL, CAT], BF16, isOutput=False)
    wvT = nc.declare_dram_parameter("wvT", [D_MODEL, CAT], BF16, isOutput=False)
    wtailT = nc.declare_dram_parameter("wtailT", [CAT, D_MODEL], BF16, isOutput=False)
    maskf = nc.declare_dram_parameter("maskf", [kpad], F32, isOutput=False)
    ident = nc.declare_dram_parameter("ident", [128, 128], BF16, isOutput=False)
    ones8 = nc.declare_dram_parameter("ones8", [128, HPC], BF16, isOutput=False)
    y = nc.declare_dram_parameter("y", [T, D_MODEL], F32, isOutput=True)

    with ExitStack() as ctx:
        tc = ctx.enter_context(tile.TileContext(nc))

        # ---- long-lived pools (entered first so short-lived ones stack on top)
        const = ctx.enter_context(tc.tile_pool(name="const", bufs=1))
        qk_pool = ctx.enter_context(tc.tile_pool(name="qk", bufs=1))
        vaug_pool = ctx.enter_context(tc.tile_pool(name="vaug", bufs=1))

        identity = const.tile([128, 128], BF16)
        nc.sync.dma_start(out=identity, in_=ident[:, :])

        # per-key-block additive exp bias: 0 for kept keys, -8e9 for pads
        maskb = const.tile([128, NKB], F32)
        nc.sync.dma_start(out=maskb, in_=maskf.rearrange("(j p) -> p j", p=128))

        # persistent intermeds
        # q/k of 2 heads per tile: rows [h0 d64 | h1 d64]
        qts = [qk_pool.tile([128, T], BF16, tag=f"qt{j}", name=f"qt{j}")
               for j in range(NPAIR)]
        kts = [qk_pool.tile([128, kpad], BF16, tag=f"kt{j}", name=f"kt{j}")
               for j in range(NPAIR)]
        # V augmented with ones column: [key-block][128, head, 65]
        vaugs = [vaug_pool.tile([128, HPC, D_HEAD + 1], BF16,
                                tag=f"va{t}", name=f"va{t}")
                 for t in range(NKB)]

        # alternate PSUM->SBUF copies between DVE and ACT so neither engine
        # serializes the PE in the projection phase
        _cp = [0]

        def copy_alt(out, in_):
            if _cp[0] % 2 == 0:
                nc.vector.tensor_copy(out=out, in_=in_)
            else:
                nc.scalar.activation(
                    out=out, in_=in_, func=mybir.ActivationFunctionType.Copy)
            _cp[0] += 1

        # x + projection weights stay resident for the whole kernel so the
        # q/k projections of pairs 1-3 can interleave into phase 2 (they are
        # the PE's filler work while ACT computes exp).
        xw_pool = ctx.enter_context(tc.tile_pool(name="xw", bufs=1))
        qkps = ctx.enter_context(tc.tile_pool(name="qkps", bufs=1, space="PSUM"))

        # ---- phase 1: V projection + pair-0 q/k projection
        with tc.tile_pool(name="vps", bufs=4, space="PSUM") as vps:
            xks = [xw_pool.tile([128, kpad], BF16, tag=f"xk{kc}", name=f"xk{kc}")
                   for kc in range(KC)]
            for kc in range(KC):
                nc.sync.dma_start(out=xks[kc],
                                  in_=xkT[kc * 128:(kc + 1) * 128, :])
            wvs = [xw_pool.tile([128, CAT], BF16, tag=f"wv{kc}", name=f"wv{kc}")
                   for kc in range(KC)]
            for kc in range(KC):
                nc.sync.dma_start(out=wvs[kc],
                                  in_=wvT[kc * 128:(kc + 1) * 128, :])
            xqs = [xw_pool.tile([128, T], BF16, tag=f"xq{kc}", name=f"xq{kc}")
                   for kc in range(KC)]
            for kc in range(KC):
                nc.sync.dma_start(out=xqs[kc],
                                  in_=xT[kc * 128:(kc + 1) * 128, :])
            wqs = [xw_pool.tile([128, KC, 128], BF16, tag=f"wq{j}", name=f"wq{j}")
                   for j in range(NPAIR)]
            wks = [xw_pool.tile([128, KC, 128], BF16, tag=f"wk{j}", name=f"wk{j}")
                   for j in range(NPAIR)]
            for j in range(NPAIR):
                nc.sync.dma_start(
                    out=wqs[j],
                    in_=wqT.rearrange("(kc p) c -> p kc c", p=128)[
                        :, :, j * 128:(j + 1) * 128])
                nc.sync.dma_start(
                    out=wks[j],
                    in_=wkT.rearrange("(kc p) c -> p kc c", p=128)[
                        :, :, j * 128:(j + 1) * 128])

            # V projection over compacted keys: V[key, cat] = xk @ Wv^T
            for tb in range(NKB):
                vp = vps.tile([128, CAT], F32, tag="vp", name="vp")
                for kc in range(KC):
                    nc.tensor.matmul(
                        vp,
                        xks[kc][:, tb * 128:(tb + 1) * 128],
                        wvs[kc],
                        start=(kc == 0), stop=(kc == KC - 1),
                    )
                va = vaugs[tb]
                nc.sync.dma_start(
                    out=va[:, :, D_HEAD:D_HEAD + 1], in_=ones8[:, :])
                copy_alt(va[:, :, 0:D_HEAD],
                         vp.rearrange("p (h d) -> p h d", h=HPC))

            # Q projection (full T) and K projection (kpad), per head pair:
            # out rows = [q(2j) 64 | q(2j+1) 64] so one [128, chunk] copy
            # moves both heads at once.  Only pair 0 runs in phase 1; pairs
            # 1-3 are emitted chunk-by-chunk inside phase 2.
            def pair_chunks(j):
                return ([("q", j, c * 512, 512) for c in range(T // 512)]
                        + [("k", j, c0, w) for (c0, w) in KCH])

            def emit_chunk(spec, dve_only=False):
                kind, j, c0, w = spec
                qp = qkps.tile([128, 512], F32, tag="qp", name="qp")
                srcs = xqs if kind == "q" else xks
                wsrc = wqs[j] if kind == "q" else wks[j]
                dst = qts[j] if kind == "q" else kts[j]
                for kc in range(KC):
                    nc.tensor.matmul(
                        qp[:, 0:w],
                        wsrc[:, kc, :],
                        srcs[kc][:, c0:c0 + w],
                        start=(kc == 0), stop=(kc == KC - 1),
                    )
                if dve_only:
                    nc.vector.tensor_copy(out=dst[:, c0:c0 + w],
                                          in_=qp[:, 0:w])
                else:
                    copy_alt(dst[:, c0:c0 + w], qp[:, 0:w])

            for spec in pair_chunks(0):
                emit_chunk(spec)

        # ---- phase 2: attention per head, q in two halves
        num_pool = ctx.enter_context(tc.tile_pool(name="num", bufs=1))
        # stacked normalized attn^T: 2 heads per tile (cat rows)
        nums = [num_pool.tile([128, T], BF16, tag=f"nm{j}", name=f"nm{j}")
                for j in range(NPAIR)]
        with tc.tile_pool(name="p_sb", bufs=5) as p_pool, \
             tc.tile_pool(name="av_sb", bufs=3) as avsb_pool, \
             tc.tile_pool(name="r_sb", bufs=4) as r_pool, \
             tc.tile_pool(name="at_sb", bufs=2) as at_pool, \
             tc.tile_pool(name="stps", bufs=2, space="PSUM") as stps, \
             tc.tile_pool(name="avps", bufs=1, space="PSUM") as avps, \
             tc.tile_pool(name="tps", bufs=1, space="PSUM") as tps:

            def warm_keeper():
                dmy = tps.tile([128, 128], F32, tag="tp", name="dmy")
                nc.tensor.matmul(dmy, identity, identity, start=True, stop=True)

            # Software-pipelined emission: within a unit (head, q-half) the
            # PE stream is ST(0),ST(1),...,ST(kb),AV(kb-LAG),... so the PE
            # always has a queued matmul while ACT computes exp; the
            # normalize (transpose) work of the previous unit is emitted
            # early in the next unit to fill the exp-latency window.

            def normalize_unit(av_sb, ap_tile, r0):
                for tb in range(QH // 128):
                    t1 = tps.tile([128, 128], BF16, tag="tp", name="t1")
                    nc.tensor.transpose(
                        t1[:, 0:D_HEAD + 1],
                        av_sb[:, tb * 128:(tb + 1) * 128],
                        identity[0:D_HEAD + 1, 0:D_HEAD + 1],
                    )
                    r_sb = r_pool.tile([128, 1], F32, tag="r", name="r_sb")
                    nc.vector.reciprocal(out=r_sb, in_=t1[:, D_HEAD:D_HEAD + 1])
                    nc.vector.tensor_scalar_mul(
                        ap_tile[:, tb, r0:r0 + 64], t1[:, 0:D_HEAD], r_sb)

            def flush_pair(aps, j):
                for half in range(2):
                    q0 = half * QH
                    for tb in range(QH // 128):
                        t2 = tps.tile([128, 128], BF16, tag="tp", name="t2")
                        nc.tensor.transpose(t2, aps[half][:, tb, :], identity)
                        nc.vector.tensor_copy(
                            out=nums[j][:, q0 + tb * 128:q0 + (tb + 1) * 128],
                            in_=t2,
                        )

            pending_norm = None   # (av_sb, ap_tile, r0)
            pending_pair = None   # (aps, j)
            cur_aps = None
            chunk_queue = []
            for pair in range(NPAIR):
                if pair + 1 < NPAIR:
                    chunk_queue.extend(pair_chunks(pair + 1))
                # token-major normalized attn for the head pair, per q-half:
                # [tok-part, tok-blk, cat(2 heads x 64)]
                cur_aps = [at_pool.tile([128, QH // 128, 128], BF16,
                                        tag=f"ap{hf}", name=f"ap{hf}")
                           for hf in range(2)]
                for sub in range(2):
                    h = 2 * pair + sub
                    r0 = sub * 64
                    qt = qts[pair][r0:r0 + 64, :]
                    kt = kts[pair][r0:r0 + 64, :]
                    for half in range(2):
                        q0 = half * QH
                        avp = avps.tile([D_HEAD + 1, QH], F32, tag="avp",
                                        name="avp")
                        p_tiles = {}

                        def emit_st_exp(kb):
                            stp = stps.tile([128, QH], F32, tag="stp",
                                            name="stp")
                            for n in range(QH // 512):
                                nc.tensor.matmul(
                                    stp[:, n * 512:(n + 1) * 512],
                                    kt[:, kb * 128:(kb + 1) * 128],
                                    qt[:, q0 + n * 512:q0 + (n + 1) * 512],
                                    start=True, stop=True,
                                )
                            p_sb = p_pool.tile([128, QH], BF16, tag="p",
                                               name="p_sb")
                            nc.scalar.activation(
                                out=p_sb, in_=stp,
                                func=mybir.ActivationFunctionType.Exp,
                                bias=maskb[:, kb:kb + 1], scale=0.125,
                            )
                            p_tiles[kb] = p_sb

                        def emit_av(kb):
                            p_sb = p_tiles.pop(kb)
                            for n in range(QH // 512):
                                nc.tensor.matmul(
                                    avp[:, n * 512:(n + 1) * 512],
                                    vaugs[kb][:, h, :],
                                    p_sb[:, n * 512:(n + 1) * 512],
                                    start=(kb == 0), stop=(kb == NKB - 1),
                                )

                        for kb in range(LAG):
                            if kb % 2 == 0:
                                warm_keeper()
                            emit_st_exp(kb)
                        # fill the exp latency with deferred PE work:
                        # previous unit's normalize/flush + up to 2 q/k
                        # projection chunks of an upcoming head pair
                        if pending_norm is not None:
                            normalize_unit(*pending_norm)
                            pending_norm = None
                        if pending_pair is not None:
                            flush_pair(*pending_pair)
                            pending_pair = None
                        for _ in range(2):
                            if chunk_queue:
                                emit_chunk(chunk_queue.pop(0), dve_only=True)
                        for kb in range(LAG, NKB):
                            if kb % 2 == 0:
                                warm_keeper()
                            emit_st_exp(kb)
                            emit_av(kb - LAG)
                        for kb in range(NKB - LAG, NKB):
                            emit_av(kb)
                        av_sb = avsb_pool.tile([D_HEAD + 1, QH], BF16,
                                               tag="avsb", name="av_sb")
                        nc.vector.tensor_copy(out=av_sb, in_=avp)
                        pending_norm = (av_sb, cur_aps[half], r0)
                pending_pair = (cur_aps, pair)
            # drain the pipeline
            if pending_norm is not None:
                normalize_unit(*pending_norm)
            if pending_pair is not None:
                flush_pair(*pending_pair)

        # ---- phase 3: tail matmul  y[tok, out] = attn_cat @ wtailT
        with tc.tile_pool(name="wt", bufs=1) as wt_pool, \
             tc.tile_pool(name="y_sb", bufs=3) as y_pool, \
             tc.tile_pool(name="yps", bufs=2, space="PSUM") as yps, \
             tc.tile_pool(name="dps3", bufs=1, space="PSUM") as dps3:

            def warm_keeper3():
                dmy3 = dps3.tile([128, 128], F32, tag="dmy3", name="dmy3")
                nc.tensor.matmul(dmy3, identity, identity, start=True, stop=True)
            wts = [wt_pool.tile([128, D_MODEL], BF16, tag=f"wt{c}", name=f"wt{c}")
                   for c in range(CAT // 128)]
            for c in range(CAT // 128):
                nc.sync.dma_start(out=wts[c], in_=wtailT[c * 128:(c + 1) * 128, :])
            for tb in range(NTB):
                warm_keeper3()
                yp = yps.tile([128, D_MODEL], F32, tag="yp")
                for n in range(D_MODEL // 512):
                    for c in range(CAT // 128):
                        nc.tensor.matmul(
                            yp[:, n * 512:(n + 1) * 512],
                            nums[c][:, tb * 128:(tb + 1) * 128],
                            wts[c][:, n * 512:(n + 1) * 512],
                            start=(c == 0), stop=(c == CAT // 128 - 1),
                        )
                y_sb = y_pool.tile([128, D_MODEL], F32, tag="ys")
                nc.vector.tensor_copy(out=y_sb, in_=yp)
                nc.sync.dma_start(out=y[tb * 128:(tb + 1) * 128, :], in_=y_sb)

    if split_waits:
        split_excess_waits(nc)
    return nc


_NC_CACHE = {}


def _get_nc(kpad):
    if kpad not in _NC_CACHE:
        _NC_CACHE[kpad] = build_nc(kpad)
    return _NC_CACHE[kpad]


def _plan(x, mask, w_qkv, w_tail):
    """Compute KPAD from the mask and shard full inputs into 8 core maps."""
    bf = mybir.dt.np(BF16)
    x = np.asarray(x, dtype=np.float32)
    mask = np.asarray(mask, dtype=np.int32)
    w_qkv = np.asarray(w_qkv, dtype=np.float32)
    w_tail = np.asarray(w_tail, dtype=np.float32)

    idxs = [np.flatnonzero(mask[b]) for b in range(BN)]
    nk_max = max(len(i) for i in idxs)
    kpad = max(128, -(-nk_max // 128) * 128)

    # per-batch compacted k/v-side inputs
    xTs, xkTs, maskfs = [], [], []
    for b in range(BN):
        idx = idxs[b]
        xkb = np.zeros((kpad, D_MODEL), dtype=np.float32)
        xkb[:len(idx)] = x[b][idx]
        mf = np.full(kpad, -8e9, dtype=np.float32)
        mf[:len(idx)] = 0.0
        xTs.append(np.ascontiguousarray(x[b].T).astype(bf))
        xkTs.append(np.ascontiguousarray(xkb.T).astype(bf))
        maskfs.append(mf)

    w3 = w_qkv.reshape(N_HEAD, 3, D_HEAD, D_MODEL)  # [head, qkv, d, dmodel]
    in_maps = []
    for c in range(8):
        b, hg = divmod(c, 2)
        H = range(hg * HPC, (hg + 1) * HPC)
        wq = np.concatenate([w3[h, 0] for h in H], axis=0)  # [512, 1024]
        wk = np.concatenate([w3[h, 1] for h in H], axis=0)
        wv = np.concatenate([w3[h, 2] for h in H], axis=0)
        wt = w_tail[:, hg * CAT:(hg + 1) * CAT]  # [1024, 512]
        in_maps.append({
            "ident": np.eye(128, dtype=bf),
            "ones8": np.ones((128, HPC), dtype=bf),
            "xT": xTs[b],
            "xkT": xkTs[b],
            "maskf": maskfs[b],
            "wqT": np.ascontiguousarray(wq.T).astype(bf),
            "wkT": np.ascontiguousarray(wk.T).astype(bf),
            "wvT": np.ascontiguousarray(wv.T).astype(bf),
            "wtailT": np.ascontiguousarray(wt.T).astype(bf),
        })
    return kpad, in_maps


def kernel(x, mask, w_qkv, w_tail, b_tail):
    kpad, in_maps = _plan(x, mask, w_qkv, w_tail)
    nc = _get_nc(kpad)
    last_err = None
    for _attempt in range(3):
        try:
            res = run_bass_kernel_spmd(nc, in_maps, list(range(8))).results
            break
        except Exception as e:  # transient device/runtime errors: retry
            last_err = e
            _time.sleep(3.0)
    else:
        raise last_err
    out = np.empty((BN, T, D_MODEL), dtype=np.float32)
    b_tail = np.asarray(b_tail, dtype=np.float32)
    for b in range(BN):
        out[b] = res[2 * b]["y"] + res[2 * b + 1]["y"] + b_tail
    return out


# revision 25
# speedup vs baseline: 1.2442x; 1.2442x over previous
"""Multi-head attention Trainium2 kernel, 8-way sharded, mask-compacted keys.

Problem: x[4,2048,1024] -> qkv proj (w_qkv [3072,1024]) -> 16-head attention
with key-padding mask -> tail proj (w_tail [1024,1024]) + b_tail.

Sharding: 8 shards = 4 batches x 2 head-groups (8 heads each). Each core
computes, for its (batch b, head-group hg):
  - q projection of x[b] (all T tokens) for its 8 heads
  - k/v projections of the mask-COMPACTED tokens of x[b] (keys with mask=0
    contribute exp(-inf)=0 to softmax, so they are dropped host-side and
    the key axis padded to KPAD, a multiple of 128; pads get bias -8e9)
  - [T x KPAD] masked attention per head
  - partial tail matmul y_part = attn_cat @ w_tail[:, cat_slice].T
Host unshards: out[b] = y_part[2b] + y_part[2b+1] + b_tail.  No collectives.

Engine strategy (trace-driven):
  - phase 1 (projections) and phase 3 (tail) are PE-dense: bf16 operands
    (1 cyc/row, background weight loads), PSUM->SBUF copies alternate
    between DVE and ACT so neither serializes the PE.
  - phase 2 (attention) is paced by ACT exp ([128,1024] tiles); PE operands
    stay float32r: the serial 4-byte weight load pads PE occupancy to
    ~match ACT, keeping the HAM clock at 8/8 (bf16 here made the PE idle
    23% per kb and the HAM halved the clock for ~180us).
"""

import time as _time

import numpy as np
from contextlib import ExitStack

import concourse.bass as bass
import concourse.mybir as mybir
import concourse.tile as tile
from concourse.bass_utils import run_bass_kernel_spmd

# ---------------------------------------------------------------------------
# walrus in this env accepts at most 2 sync waits per instruction; Tile's
# scheduler emits up to 10. Post-pass: peel excess waits onto same-engine
# NoOps inserted immediately before the offending instruction (same engine
# stream position => identical synchronization semantics).
MAX_WAITS = 1


def split_excess_waits(nc):
    for fn in nc.m.functions:
        for bb in fn.blocks:
            insts = list(bb.instructions)
            out = []
            changed = False
            for inst in insts:
                si = inst.sync_info
                waits = list(si.on_wait) if si is not None else []
                if len(waits) > MAX_WAITS:
                    extra = waits[:-MAX_WAITS]
                    for ci in range(0, len(extra), MAX_WAITS):
                        chunk = extra[ci:ci + MAX_WAITS]
                        nop = mybir.InstNoOp(
                            name=f"{inst.name}-ws{ci}", ins=[], outs=[])
                        nop.engine = inst.engine
                        nop.sync_info = mybir.SyncInfo(
                            on_wait=chunk, on_update=[])
                        out.append(nop)
                    inst.sync_info = mybir.SyncInfo(
                        on_wait=waits[-MAX_WAITS:],
                        on_update=list(si.on_update))
                    changed = True
                out.append(inst)
            if changed:
                bb.instructions = out
# ---------------------------------------------------------------------------

D_MODEL = 1024
N_HEAD = 16
D_HEAD = 64
BN, T = 4, 2048
HPC = 8                      # heads per core
NPAIR = HPC // 2             # head pairs (q/k tiles hold 2 heads)
CAT = HPC * D_HEAD           # 512 per-core tail contraction
NTB = T // 128               # 16 query-token blocks
QH = T // 2                  # 1024, q processed in two halves
KC = D_MODEL // 128          # 8 contraction chunks
F32 = mybir.dt.float32
F32R = mybir.dt.float32r
BF16 = mybir.dt.bfloat16
I32 = mybir.dt.int32


def build_nc(kpad, split_waits=True):
    assert kpad % 128 == 0 and 128 <= kpad <= T
    NKB = kpad // 128        # key blocks
    LAG = min(4, NKB - 1) if NKB > 1 else 0
    # k-projection chunks: (start, width), width 512 or the tail remainder
    KCH = [(c * 512, min(512, kpad - c * 512))
           for c in range((kpad + 511) // 512)]

    nc = bass.Bass()
    xT = nc.declare_dram_parameter("xT", [D_MODEL, T], BF16, isOutput=False)
    xkT = nc.declare_dram_parameter("xkT", [D_MODEL, kpad], BF16, isOutput=False)
    wqT = nc.declare_dram_parameter("wqT", [D_MODEL, CAT], BF16, isOutput=False)
    wkT = nc.declare_dram_parameter("wkT", [D_MODEL, CAT], BF16, isOutput=False)
    wvT = nc.declare_dram_parameter("wvT", [D_MODEL, CAT], BF16, isOutput=False)
    wtailT = nc.declare_dram_parameter("wtailT", [CAT, D_MODEL], BF16, isOutput=False)
    maskf = nc.declare_dram_parameter("maskf", [kpad], F32, isOutput=False)
    ident = nc.declare_dram_parameter("ident", [128, 128], F32, isOutput=False)
    ones8 = nc.declare_dram_parameter("ones8", [128, HPC], BF16, isOutput=False)
    y = nc.declare_dram_parameter("y", [T, D_MODEL], F32, isOutput=True)

    with ExitStack() as ctx:
        tc = ctx.enter_context(tile.TileContext(nc))

        # ---- long-lived pools (entered first so short-lived ones stack on top)
        const = ctx.enter_context(tc.tile_pool(name="const", bufs=1))
        qk_pool = ctx.enter_context(tc.tile_pool(name="qk", bufs=1))
        vaug_pool = ctx.enter_context(tc.tile_pool(name="vaug", bufs=1))

        identity = const.tile([128, 128], F32)
        nc.sync.dma_start(out=identity, in_=ident[:, :])

        # per-key-block additive exp bias: 0 for kept keys, -8e9 for pads
        maskb = const.tile([128, NKB], F32)
        nc.sync.dma_start(out=maskb, in_=maskf.rearrange("(j p) -> p j", p=128))

        # persistent intermeds
        # q/k of 2 heads per tile: rows [h0 d64 | h1 d64]
        qts = [qk_pool.tile([128, T], BF16, tag=f"qt{j}", name=f"qt{j}")
               for j in range(NPAIR)]
        kts = [qk_pool.tile([128, kpad], BF16, tag=f"kt{j}", name=f"kt{j}")
               for j in range(NPAIR)]
        # V augmented with ones column: [key-block][128, head, 65]
        vaugs = [vaug_pool.tile([128, HPC, D_HEAD + 1], BF16,
                                tag=f"va{t}", name=f"va{t}")
                 for t in range(NKB)]

        # alternate PSUM->SBUF copies between DVE and ACT so neither engine
        # serializes the PE in the projection phase
        _cp = [0]

        def copy_alt(out, in_):
            if _cp[0] % 2 == 0:
                nc.vector.tensor_copy(out=out, in_=in_)
            else:
                nc.scalar.activation(
                    out=out, in_=in_, func=mybir.ActivationFunctionType.Copy)
            _cp[0] += 1

        # x + projection weights stay resident for the whole kernel so the
        # q/k projections of pairs 1-3 can interleave into phase 2 (they are
        # the PE's filler work while ACT computes exp).
        xw_pool = ctx.enter_context(tc.tile_pool(name="xw", bufs=1))
        qkps = ctx.enter_context(tc.tile_pool(name="qkps", bufs=1, space="PSUM"))

        # ---- phase 1: V projection + pair-0 q/k projection
        with tc.tile_pool(name="vps", bufs=4, space="PSUM") as vps:
            xks = [xw_pool.tile([128, kpad], BF16, tag=f"xk{kc}", name=f"xk{kc}")
                   for kc in range(KC)]
            for kc in range(KC):
                nc.sync.dma_start(out=xks[kc],
                                  in_=xkT[kc * 128:(kc + 1) * 128, :])
            wvs = [xw_pool.tile([128, CAT], BF16, tag=f"wv{kc}", name=f"wv{kc}")
                   for kc in range(KC)]
            for kc in range(KC):
                nc.sync.dma_start(out=wvs[kc],
                                  in_=wvT[kc * 128:(kc + 1) * 128, :])
            xqs = [xw_pool.tile([128, T], BF16, tag=f"xq{kc}", name=f"xq{kc}")
                   for kc in range(KC)]
            for kc in range(KC):
                nc.sync.dma_start(out=xqs[kc],
                                  in_=xT[kc * 128:(kc + 1) * 128, :])
            wqs = [xw_pool.tile([128, KC, 128], BF16, tag=f"wq{j}", name=f"wq{j}")
                   for j in range(NPAIR)]
            wks = [xw_pool.tile([128, KC, 128], BF16, tag=f"wk{j}", name=f"wk{j}")
                   for j in range(NPAIR)]
            for j in range(NPAIR):
                nc.sync.dma_start(
                    out=wqs[j],
                    in_=wqT.rearrange("(kc p) c -> p kc c", p=128)[
                        :, :, j * 128:(j + 1) * 128])
                nc.sync.dma_start(
                    out=wks[j],
                    in_=wkT.rearrange("(kc p) c -> p kc c", p=128)[
                        :, :, j * 128:(j + 1) * 128])

            # V projection over compacted keys: V[key, cat] = xk @ Wv^T
            for tb in range(NKB):
                vp = vps.tile([128, CAT], F32, tag="vp", name="vp")
                for kc in range(KC):
                    nc.tensor.matmul(
                        vp,
                        xks[kc][:, tb * 128:(tb + 1) * 128],
                        wvs[kc],
                        start=(kc == 0), stop=(kc == KC - 1),
                    )
                va = vaugs[tb]
                nc.sync.dma_start(
                    out=va[:, :, D_HEAD:D_HEAD + 1], in_=ones8[:, :])
                copy_alt(va[:, :, 0:D_HEAD],
                         vp.rearrange("p (h d) -> p h d", h=HPC))

            # Q projection (full T) and K projection (kpad), per head pair:
            # out rows = [q(2j) 64 | q(2j+1) 64] so one [128, chunk] copy
            # moves both heads at once.  Only pair 0 runs in phase 1; pairs
            # 1-3 are emitted chunk-by-chunk inside phase 2.
            def pair_chunks(j):
                return ([("q", j, c * 512, 512) for c in range(T // 512)]
                        + [("k", j, c0, w) for (c0, w) in KCH])

            def emit_chunk(spec, dve_only=False):
                kind, j, c0, w = spec
                qp = qkps.tile([128, 512], F32, tag="qp", name="qp")
                srcs = xqs if kind == "q" else xks
                wsrc = wqs[j] if kind == "q" else wks[j]
                dst = qts[j] if kind == "q" else kts[j]
                for kc in range(KC):
                    nc.tensor.matmul(
                        qp[:, 0:w],
                        wsrc[:, kc, :],
                        srcs[kc][:, c0:c0 + w],
                        start=(kc == 0), stop=(kc == KC - 1),
                    )
                if dve_only:
                    nc.vector.tensor_copy(out=dst[:, c0:c0 + w],
                                          in_=qp[:, 0:w])
                else:
                    copy_alt(dst[:, c0:c0 + w], qp[:, 0:w])

            for spec in pair_chunks(0):
                emit_chunk(spec)

        # ---- phase 2: attention per head PAIR, q in four quarters.
        # The two heads of a pair live at SBUF partitions 0-63 / 64-127 of
        # qts/kts, so their K=64 S^T matmuls land on disjoint PE row groups
        # and execute CONCURRENTLY (measured 152ns vs 467ns per N=512 mm).
        # Both heads' scores for one (kb, quarter) go into one [128,2,512]
        # PSUM tile so a single 1024-wide exp covers them.
        num_pool = ctx.enter_context(tc.tile_pool(name="num", bufs=1))
        # stacked normalized attn^T: 2 heads per tile (cat rows)
        nums = [num_pool.tile([128, T], BF16, tag=f"nm{j}", name=f"nm{j}")
                for j in range(NPAIR)]
        NQTR = T // 512          # 4 q-quarters
        with tc.tile_pool(name="p_sb", bufs=5) as p_pool, \
             tc.tile_pool(name="av_sb", bufs=4) as avsb_pool, \
             tc.tile_pool(name="r_sb", bufs=4) as r_pool, \
             tc.tile_pool(name="at_sb", bufs=2) as at_pool, \
             tc.tile_pool(name="stps", bufs=2, space="PSUM") as stps, \
             tc.tile_pool(name="avps", bufs=1, space="PSUM") as avps, \
             tc.tile_pool(name="tps", bufs=1, space="PSUM") as tps:

            def normalize_unit(av_sb, ap_tile, r0, qtr):
                for i in range(4):
                    tb = qtr * 4 + i
                    t1 = tps.tile([128, 128], F32, tag="tp", name="t1")
                    nc.tensor.transpose(
                        t1[:, 0:D_HEAD + 1],
                        av_sb[:, i * 128:(i + 1) * 128],
                        identity[0:D_HEAD + 1, 0:D_HEAD + 1],
                    )
                    r_sb = r_pool.tile([128, 1], F32, tag="r", name="r_sb")
                    nc.vector.reciprocal(out=r_sb, in_=t1[:, D_HEAD:D_HEAD + 1])
                    nc.vector.tensor_scalar_mul(
                        ap_tile[:, tb, r0:r0 + 64], t1[:, 0:D_HEAD], r_sb)

            def flush_pair(ap_tile, j):
                for tb in range(NTB):
                    t2 = tps.tile([128, 128], F32, tag="tp", name="t2")
                    nc.tensor.transpose(t2, ap_tile[:, tb, :], identity)
                    nc.vector.tensor_copy(
                        out=nums[j][:, tb * 128:(tb + 1) * 128],
                        in_=t2,
                    )

            # projection work of upcoming pairs, flattened to single-mm ops
            # popped 2 per kb slot as PE filler
            chunk_ops = {}   # pair j -> list of closures

            def queue_chunk_ops(j):
                ops = []
                for spec in pair_chunks(j):
                    kind, jj, c0, w = spec
                    cell = {}
                    for kc in range(KC):
                        def mm(kc=kc, kind=kind, jj=jj, c0=c0, w=w, cell=cell):
                            if kc == 0:
                                cell["qp"] = qkps.tile([128, 512], F32,
                                                       tag="qp", name="qp")
                            srcs = xqs if kind == "q" else xks
                            wsrc = wqs[jj] if kind == "q" else wks[jj]
                            nc.tensor.matmul(
                                cell["qp"][:, 0:w],
                                wsrc[:, kc, :],
                                srcs[kc][:, c0:c0 + w],
                                start=(kc == 0), stop=(kc == KC - 1),
                            )
                        ops.append(mm)

                    def cp(kind=kind, jj=jj, c0=c0, w=w, cell=cell):
                        dst = qts[jj] if kind == "q" else kts[jj]
                        nc.vector.tensor_copy(out=dst[:, c0:c0 + w],
                                              in_=cell["qp"][:, 0:w])
                    ops.append(cp)
                chunk_ops[j] = ops

            def pop_chunk_ops(j, n):
                lst = chunk_ops.get(j)
                for _ in range(n):
                    if lst:
                        lst.pop(0)()

            pending_norms = []    # (av_sb, ap_tile, r0, qtr)
            pending_pair = None   # (ap_tile, j)
            for pair in range(NPAIR):
                # leftover projection work for THIS pair must finish now
                for op in chunk_ops.pop(pair, []):
                    op()
                if pair + 1 < NPAIR:
                    queue_chunk_ops(pair + 1)
                h0, h1 = 2 * pair, 2 * pair + 1
                # token-major normalized attn for the pair:
                # [tok-part, tok-blk, cat(2 heads x 64)]
                ap_tile = at_pool.tile([128, NTB, 128], F32,
                                       tag="ap", name="ap")
                for qtr in range(NQTR):
                    q0 = qtr * 512
                    avp0 = avps.tile([D_HEAD + 1, 512], F32, tag="av0",
                                     name="avp0")
                    avp1 = avps.tile([D_HEAD + 1, 512], F32, tag="av1",
                                     name="avp1")
                    p_tiles = {}

                    def emit_st_exp(kb):
                        stp2 = stps.tile([128, 2, 512], F32, tag="stp",
                                         name="stp2")
                        nc.tensor.matmul(
                            stp2[:, 0, :],
                            kts[pair][0:64, kb * 128:(kb + 1) * 128],
                            qts[pair][0:64, q0:q0 + 512],
                            start=True, stop=True,
                        )
                        nc.tensor.matmul(
                            stp2[:, 1, :],
                            kts[pair][64:128, kb * 128:(kb + 1) * 128],
                            qts[pair][64:128, q0:q0 + 512],
                            start=True, stop=True,
                        )
                        p2 = p_pool.tile([128, 2, 512], BF16, tag="p",
                                         name="p2")
                        nc.scalar.activation(
                            out=p2, in_=stp2,
                            func=mybir.ActivationFunctionType.Exp,
                            bias=maskb[:, kb:kb + 1], scale=0.125,
                        )
                        p_tiles[kb] = p2

                    def emit_av(kb):
                        p2 = p_tiles.pop(kb)
                        nc.tensor.matmul(
                            avp0, vaugs[kb][:, h0, :], p2[:, 0, :],
                            start=(kb == 0), stop=(kb == NKB - 1),
                        )
                        nc.tensor.matmul(
                            avp1, vaugs[kb][:, h1, :], p2[:, 1, :],
                            start=(kb == 0), stop=(kb == NKB - 1),
                        )

                    for kb in range(LAG):
                        emit_st_exp(kb)
                    # fill the exp latency with deferred PE work
                    while pending_norms:
                        normalize_unit(*pending_norms.pop(0))
                    if pending_pair is not None:
                        flush_pair(*pending_pair)
                        pending_pair = None
                    for kb in range(LAG, NKB):
                        emit_st_exp(kb)
                        emit_av(kb - LAG)
                        pop_chunk_ops(pair + 1, 2)
                    for kb in range(NKB - LAG, NKB):
                        emit_av(kb)
                    av_sb0 = avsb_pool.tile([D_HEAD + 1, 512], F32,
                                            tag="avsb", name="av_sb0")
                    nc.vector.tensor_copy(out=av_sb0, in_=avp0)
                    av_sb1 = avsb_pool.tile([D_HEAD + 1, 512], F32,
                                            tag="avsb", name="av_sb1")
                    nc.vector.tensor_copy(out=av_sb1, in_=avp1)
                    pending_norms.append((av_sb0, ap_tile, 0, qtr))
                    pending_norms.append((av_sb1, ap_tile, 64, qtr))
                pending_pair = (ap_tile, pair)
            # drain the pipeline
            while pending_norms:
                normalize_unit(*pending_norms.pop(0))
            if pending_pair is not None:
                flush_pair(*pending_pair)

        # ---- phase 3: tail matmul  y[tok, out] = attn_cat @ wtailT
        with tc.tile_pool(name="wt", bufs=1) as wt_pool, \
             tc.tile_pool(name="y_sb", bufs=3) as y_pool, \
             tc.tile_pool(name="yps", bufs=2, space="PSUM") as yps, \
             tc.tile_pool(name="dps3", bufs=1, space="PSUM") as dps3:

            def warm_keeper3():
                dmy3 = dps3.tile([128, 128], F32, tag="dmy3", name="dmy3")
                nc.tensor.matmul(dmy3, identity, identity, start=True, stop=True)
            wts = [wt_pool.tile([128, D_MODEL], BF16, tag=f"wt{c}", name=f"wt{c}")
                   for c in range(CAT // 128)]
            for c in range(CAT // 128):
                nc.sync.dma_start(out=wts[c], in_=wtailT[c * 128:(c + 1) * 128, :])
            for tb in range(NTB):
                warm_keeper3()
                yp = yps.tile([128, D_MODEL], F32, tag="yp")
                for n in range(D_MODEL // 512):
                    for c in range(CAT // 128):
                        nc.tensor.matmul(
                            yp[:, n * 512:(n + 1) * 512],
                            nums[c][:, tb * 128:(tb + 1) * 128],
                            wts[c][:, n * 512:(n + 1) * 512],
                            start=(c == 0), stop=(c == CAT // 128 - 1),
                        )
                y_sb = y_pool.tile([128, D_MODEL], F32, tag="ys")
                nc.vector.tensor_copy(out=y_sb, in_=yp)
                nc.sync.dma_start(out=y[tb * 128:(tb + 1) * 128, :], in_=y_sb)

    if split_waits:
        split_excess_waits(nc)
    return nc


_NC_CACHE = {}


def _get_nc(kpad):
    if kpad not in _NC_CACHE:
        _NC_CACHE[kpad] = build_nc(kpad)
    return _NC_CACHE[kpad]


def _plan(x, mask, w_qkv, w_tail):
    """Compute KPAD from the mask and shard full inputs into 8 core maps."""
    bf = mybir.dt.np(BF16)
    x = np.asarray(x, dtype=np.float32)
    mask = np.asarray(mask, dtype=np.int32)
    w_qkv = np.asarray(w_qkv, dtype=np.float32)
    w_tail = np.asarray(w_tail, dtype=np.float32)

    idxs = [np.flatnonzero(mask[b]) for b in range(BN)]
    nk_max = max(len(i) for i in idxs)
    kpad = max(128, -(-nk_max // 128) * 128)

    # per-batch compacted k/v-side inputs
    xTs, xkTs, maskfs = [], [], []
    for b in range(BN):
        idx = idxs[b]
        xkb = np.zeros((kpad, D_MODEL), dtype=np.float32)
        xkb[:len(idx)] = x[b][idx]
        mf = np.full(kpad, -8e9, dtype=np.float32)
        mf[:len(idx)] = 0.0
        xTs.append(np.ascontiguousarray(x[b].T).astype(bf))
        xkTs.append(np.ascontiguousarray(xkb.T).astype(bf))
        maskfs.append(mf)

    w3 = w_qkv.reshape(N_HEAD, 3, D_HEAD, D_MODEL)  # [head, qkv, d, dmodel]
    in_maps = []
    for c in range(8):
        b, hg = divmod(c, 2)
        H = range(hg * HPC, (hg + 1) * HPC)
        wq = np.concatenate([w3[h, 0] for h in H], axis=0)  # [512, 1024]
        wk = np.concatenate([w3[h, 1] for h in H], axis=0)
        wv = np.concatenate([w3[h, 2] for h in H], axis=0)
        wt = w_tail[:, hg * CAT:(hg + 1) * CAT]  # [1024, 512]
        in_maps.append({
            "ident": np.eye(128, dtype=np.float32),
            "ones8": np.ones((128, HPC), dtype=bf),
            "xT": xTs[b],
            "xkT": xkTs[b],
            "maskf": maskfs[b],
            "wqT": np.ascontiguousarray(wq.T).astype(bf),
            "wkT": np.ascontiguousarray(wk.T).astype(bf),
            "wvT": np.ascontiguousarray(wv.T).astype(bf),
            "wtailT": np.ascontiguousarray(wt.T).astype(bf),
        })
    return kpad, in_maps


def kernel(x, mask, w_qkv, w_tail, b_tail):
    kpad, in_maps = _plan(x, mask, w_qkv, w_tail)
    nc = _get_nc(kpad)
    last_err = None
    for _attempt in range(3):
        try:
            res = run_bass_kernel_spmd(nc, in_maps, list(range(8))).results
            break
        except Exception as e:  # transient device/runtime errors: retry
            last_err = e
            _time.sleep(3.0)
    else:
        raise last_err
    out = np.empty((BN, T, D_MODEL), dtype=np.float32)
    b_tail = np.asarray(b_tail, dtype=np.float32)
    for b in range(BN):
        out[b] = res[2 * b]["y"] + res[2 * b + 1]["y"] + b_tail
    return out
